# revision 20
# baseline (speedup 1.0000x reference)
"""Trainium2 Bass kernel: FAVOR (Performer) causal linear attention block.

Per batch element (data-parallel over 8 NeuronCores):
  c = x @ w_inp + b_inp; q,k,v = split(c)
  qf/kf = rfm_softmax(q/k, omega)             (FAVOR random feature maps)
  a     = causal_linear_attention(qf, kf, v)  (masked score matmuls)
  out   = a @ w_out + b_out
"""

import numpy as np
from contextlib import ExitStack

import concourse.bass as bass
import concourse.tile as tile
from concourse import mybir
from concourse import bass_utils
import bass_rust

F32 = mybir.dt.float32
F32R = mybir.dt.float32r
BF16 = mybir.dt.bfloat16
AF = mybir.ActivationFunctionType

B, L, E, H, Dh, F = 8, 512, 768, 12, 64, 64
O3 = 3 * E
LT = L // 128      # 4 l-chunks
ET = E // 128      # 6 e-chunks
NH2 = H // 2       # 6 head pairs
EPS = 1e-6
LN8 = 2.0794415416798357   # 0.5 * ln(F)
SCALE_D = float(Dh) ** -0.25
EPSP = EPS * (float(F) ** -0.5)

ATTN_BF16 = False  # attention-path dtype switch


def _fix_waits(nc, cap=1):
    """Walrus codegen in this toolchain allows a single sync-wait per
    instruction; hoist excess waits onto injected same-engine NoOps placed
    directly before the offender (no reordering, deadlock-free)."""
    n = 0
    for fn in nc.m.functions:
        for bb in fn.blocks:
            insts = bb.instructions
            i = 0
            while i < len(insts):
                inst = insts[i]
                si = inst.sync_info
                if si is not None:
                    ow = list(si.on_wait)
                    if len(ow) > cap:
                        excess, keep = ow[:-cap], ow[-cap:]
                        si.on_wait = keep
                        for w in excess:
                            n += 1
                            nop = bass_rust.InstNoOp(
                                name=f"waitnop_{n}",
                                engine=inst.engine,
                                sync_info=bass_rust.SyncInfo(
                                    on_wait=[w], on_update=[]),
                            )
                            insts.insert(i, nop)
                            i += 1
                i += 1
    return n


def build_nc(attn_bf16=ATTN_BF16, fix_waits=True, phases=99):
    nc = bass.Bass("TRN2", target_bir_lowering=False, debug=False, num_devices=8)
    AD = BF16 if attn_bf16 else F32R   # attn-path matmul-operand dtype
    QD = BF16 if attn_bf16 else F32    # qf dtype
    KD = BF16 if attn_bf16 else F32R   # kf dtype (K1 matmul rhs)
    WD = F32 if attn_bf16 else F32R    # w_out DMA dtype

    x_d = nc.dram_tensor("x", [L, E], F32, kind="ExternalInput").ap()
    w_inp_d = nc.dram_tensor("w_inp", [E, O3], F32R, kind="ExternalInput").ap()
    b_inp_d = nc.dram_tensor("b_inp", [O3], F32, kind="ExternalInput").ap()
    w_out_d = nc.dram_tensor("w_out", [E, E], WD, kind="ExternalInput").ap()
    b_out_d = nc.dram_tensor("b_out", [E], F32, kind="ExternalInput").ap()
    omega_d = nc.dram_tensor("omega", [F, Dh], F32, kind="ExternalInput").ap()
    ident_d = nc.dram_tensor("ident", [128, 128], F32, kind="ExternalInput").ap()
    identr_d = nc.dram_tensor("ident_r", [128, 128], F32R, kind="ExternalInput").ap()
    identa_d = nc.dram_tensor("ident_a", [128, 128], BF16, kind="ExternalInput").ap()
    maskd_d = nc.dram_tensor("mask_diag", [128, 128], AD, kind="ExternalInput").ap()
    ones_d = nc.dram_tensor("ones128", [128, 128], AD, kind="ExternalInput").ap()
    out_d = nc.dram_tensor("out", [L, E], F32, kind="ExternalOutput").ap()

    def bc(ap, p=128):
        # broadcast a 1-D DRAM AP across p partitions
        return bass.AP(tensor=ap.tensor, offset=ap.offset,
                       ap=[[0, p]] + [list(d) for d in ap.ap])

    class _PhaseCutE(Exception):
        pass
    global _PhaseCut
    _PhaseCut = _PhaseCutE
    with tile.TileContext(nc) as tc, ExitStack() as ctx:
      try:
        P = ctx.enter_context(tc.tile_pool(name="persist", bufs=1))
        wqk_p = ctx.enter_context(tc.tile_pool(name="wqk", bufs=3))
        wv_p = ctx.enter_context(tc.tile_pool(name="wv", bufs=2))
        xin_p = ctx.enter_context(tc.tile_pool(name="xin", bufs=1))
        ssub_p = ctx.enter_context(tc.tile_pool(name="ssub", bufs=3))
        bexp_p = ctx.enter_context(tc.tile_pool(name="bexp", bufs=2))
        st_p = ctx.enter_context(tc.tile_pool(name="stp", bufs=6))
        dn_p = ctx.enter_context(tc.tile_pool(name="dnp", bufs=2))
        sm_p = ctx.enter_context(tc.tile_pool(name="smp", bufs=10))
        osb_p = ctx.enter_context(tc.tile_pool(name="osb", bufs=2))
        ps = ctx.enter_context(tc.tile_pool(name="ps", bufs=8, space="PSUM"))

        cnt = [0]

        def pst(shape, dtype=F32):
            cnt[0] += 1
            return ps.tile(shape, dtype, tag="ps", name=f"pst{cnt[0]}")

        # ---------------- x load (transposes interleaved with QKV) --------
        xT = [P.tile([128, L], F32R, tag=f"xT{et}", name=f"xT{et}")
              for et in range(ET)]
        xins = []
        for lt in range(LT):
            xin = xin_p.tile([128, E], F32, tag=f"xin{lt}", name=f"xin{lt}")
            nc.gpsimd.dma_start(out=xin, in_=x_d[lt * 128:(lt + 1) * 128, :])
            xins.append(xin)
        # ---------------- constants ----------------
        ident = P.tile([128, 128], F32, tag="ident", name="ident")
        nc.gpsimd.dma_start(out=ident, in_=ident_d)
        omega_w = P.tile([128, 128], F32, tag="omega_w", name="omega_w")
        for rr_ in range(2):
            for cc_ in range(2):
                nc.gpsimd.dma_start(
                    out=omega_w[rr_ * 64:(rr_ + 1) * 64, cc_ * 64:(cc_ + 1) * 64],
                    in_=omega_d)
        identt = P.tile([128, 128], BF16 if attn_bf16 else F32R,
                        tag="identt", name="identt")
        nc.gpsimd.dma_start(out=identt, in_=identa_d if attn_bf16 else identr_d)
        maskd = P.tile([128, 128], AD, tag="maskd", name="maskd")
        nc.gpsimd.dma_start(out=maskd, in_=maskd_d)
        ones128 = P.tile([128, 128], AD, tag="ones128", name="ones128")
        nc.gpsimd.dma_start(out=ones128, in_=ones_d)

        b_inpT = P.tile([128, 12], F32, tag="b_inpT", name="b_inpT")
        nc.gpsimd.dma_start(out=b_inpT,
                          in_=b_inp_d.rearrange("(j p) -> p j", p=128)[:, 0:12])
        b_inp_v = P.tile([128, E], F32, tag="b_inp_v", name="b_inp_v")
        nc.gpsimd.dma_start(out=b_inp_v, in_=bc(b_inp_d[2 * E:3 * E]))
        b_out_sb = P.tile([128, E], F32, tag="b_out_sb", name="b_out_sb")
        nc.gpsimd.dma_start(out=b_out_sb, in_=bc(b_out_d))


        # w_out resident (reused by all 4 l-chunks)
        w_out_sb = []
        for et in range(ET):
            t = P.tile([128, E], WD, tag=f"wo{et}", name=f"wo{et}")
            nc.sync.dma_start(out=t, in_=w_out_d[et * 128:(et + 1) * 128, :])
            w_out_sb.append(t)
        if attn_bf16:
            wo_b = []
            for et in range(ET):
                t = P.tile([128, E], BF16, tag=f"wob{et}", name=f"wob{et}")
                nc.vector.tensor_copy(t, w_out_sb[et])
                wo_b.append(t)


        if phases < 1:
            raise _PhaseCut
        for et in range(ET):
            for lt in range(LT):
                p = pst([128, 128])
                nc.tensor.transpose(p, xins[lt][:, et * 128:(et + 1) * 128],
                                    ident)
                if lt % 2 == 0:
                    nc.vector.tensor_copy(xT[et][:, lt * 128:(lt + 1) * 128], p)
                else:
                    nc.scalar.copy(xT[et][:, lt * 128:(lt + 1) * 128], p)

        # ---------------- QKV: q,k transposed ----------------
        # cT[ot] [o=128, l=512]; ot 0..5 -> q channels, 6..11 -> k channels
        cT = [P.tile([128, L], F32R, tag=f"cT{ot}", name=f"cT{ot}")
              for ot in range(12)]
        for grp in range(2):  # 0: q section, 1: k section
            pcs = [pst([128, L]) for _ in range(6)]
            for et in range(ET):
                wt = wqk_p.tile([128, E], F32R, tag="wqk", name="wqk")
                nc.sync.dma_start(
                    out=wt,
                    in_=w_inp_d[et * 128:(et + 1) * 128, grp * E:(grp + 1) * E])
                for o in range(6):
                    nc.tensor.matmul(pcs[o], wt[:, o * 128:(o + 1) * 128],
                                     xT[et], start=(et == 0), stop=(et == ET - 1))
            for o in range(6):
                ot = grp * 6 + o
                nc.scalar.activation(cT[ot], pcs[o], AF.Identity,
                                     bias=b_inpT[:, ot:ot + 1], scale=1.0)

        if phases < 2:
            raise _PhaseCut
        # ---------------- QKV: v natural [l, o] ----------------
        # v stored zero-padded per head: head h lives in cols
        # [h*128 + (h%2)*64, +64) of v_pboth, rest zero -> every attn matmul
        # runs with a full [128,128] lhsT (no PE array-tiling modes)
        v_pboth = [P.tile([128, H * 128], AD, tag=f"vp{lt}", name=f"vp{lt}")
                   for lt in range(LT)]
        for lt in range(LT):
            nc.scalar.mul(v_pboth[lt][:, 0:E], b_inp_v, 0.0)
            nc.scalar.mul(v_pboth[lt][:, E:2 * E], b_inp_v, 0.0)
        for nh in range(2):
            pv = [pst([128, 384]) for _ in range(LT)]
            for et in range(ET):
                wt = wv_p.tile([128, 384], F32R, tag="wv", name="wv")
                nc.sync.dma_start(
                    out=wt,
                    in_=w_inp_d[et * 128:(et + 1) * 128,
                                2 * E + nh * 384:2 * E + (nh + 1) * 384])
                for lt in range(LT):
                    nc.tensor.matmul(pv[lt], xT[et][:, lt * 128:(lt + 1) * 128],
                                     wt, start=(et == 0), stop=(et == ET - 1))
            for lt in range(LT):
                pvr = pv[lt].rearrange("p (t x) -> p t x", x=128)
                bvr = b_inp_v[:, nh * 384:(nh + 1) * 384].rearrange(
                    "p (t x) -> p t x", x=128)
                vpr = v_pboth[lt].rearrange("p (t x) -> p t x", x=256)[
                    :, nh * 3:(nh + 1) * 3, :]
                # even heads of this half -> block offset 0; odd -> offset 192
                nc.vector.tensor_add(vpr[:, :, 0:64], pvr[:, :, 0:64],
                                     bvr[:, :, 0:64])
                nc.vector.tensor_add(vpr[:, :, 192:256], pvr[:, :, 64:128],
                                     bvr[:, :, 64:128])

        if phases < 3:
            raise _PhaseCut
        pt = pst([128, 128])
        nc.tensor.transpose(pt, omega_w, ident)
        oz = []  # oz[0]: rows 0:64 live; oz[1]: rows 64:128 live
        for par in range(2):
            t = P.tile([128, 64], F32R, tag=f"oz{par}", name=f"oz{par}")
            nc.scalar.mul(t, b_inp_v[:, 0:64], 0.0)
            half = slice(par * 64, par * 64 + 64)
            nc.scalar.mul(t[half, :], pt[half, 0:64], SCALE_D)
            oz.append(t)
        # ---------------- FAVOR feature maps ----------------
        qf = [P.tile([128, H * F], QD, tag=f"qf{lt}", name=f"qf{lt}")
              for lt in range(LT)]
        kf = [P.tile([128, H * F], KD, tag=f"kf{lt}", name=f"kf{lt}")
              for lt in range(LT)]
        for lt in range(LT):
            for qk in range(2):  # 0: q, 1: k
                sA = pst([128, 512])
                sB = pst([128, 256])
                for h in range(H):
                    lhsT = cT[qk * 6 + h // 2][:, lt * 128:(lt + 1) * 128]
                    rhs = oz[h % 2]
                    dst = (sA[:, (h % 8) * 64:(h % 8) * 64 + 64] if h < 8
                           else sB[:, (h - 8) * 64:(h - 8) * 64 + 64])
                    nc.tensor.matmul(dst, lhsT, rhs, start=True, stop=True)
                d_all = sm_p.tile([128, 12], F32, tag="d_all", name="d_all")
                m_all = sm_p.tile([128, 12], F32, tag="m_all", name="m_all")
                nc.vector.reduce_sum(d_all[:, 0:8],
                                     sA.rearrange("p (h f) -> p h f", f=64),
                                     axis=mybir.AxisListType.X)
                nc.vector.reduce_sum(d_all[:, 8:12],
                                     sB.rearrange("p (h f) -> p h f", f=64),
                                     axis=mybir.AxisListType.X)
                nc.vector.reduce_max(m_all[:, 0:8],
                                     sA.rearrange("p (h f) -> p h f", f=64),
                                     axis=mybir.AxisListType.X)
                nc.vector.reduce_max(m_all[:, 8:12],
                                     sB.rearrange("p (h f) -> p h f", f=64),
                                     axis=mybir.AxisListType.X)
                bias_all = sm_p.tile([128, 12], F32, tag="bias_all",
                                     name="bias_all")
                nc.vector.tensor_scalar(bias_all, d_all, -0.5, -LN8,
                                        op0=mybir.AluOpType.mult,
                                        op1=mybir.AluOpType.add)
                if qk == 0:
                    nc.vector.tensor_sub(bias_all, bias_all, m_all)
                else:
                    mk = sm_p.tile([128, 1], F32, tag="mk", name="mk")
                    nc.vector.reduce_max(mk, m_all, axis=mybir.AxisListType.X)
                    nc.vector.tensor_sub(bias_all, bias_all,
                                         mk.to_broadcast((128, 12)))
                bias_exp = bexp_p.tile([128, 12, 64], F32, tag="bexp",
                                       name="bexp")
                nc.gpsimd.tensor_copy(
                    bias_exp, bias_all.unsqueeze(2).broadcast_to((128, 12, 64)))
                s_sub = ssub_p.tile([128, H * F], F32, tag="ssub", name="ssub")
                nc.vector.tensor_add(s_sub[:, 0:512], sA, bias_exp[:, 0:8, :])
                nc.vector.tensor_add(s_sub[:, 512:768], sB, bias_exp[:, 8:12, :])
                dst = qf[lt] if qk == 0 else kf[lt]
                nc.scalar.activation(dst, s_sub, AF.Exp)
                if qk == 0:
                    nc.gpsimd.tensor_scalar_add(dst, dst, EPSP)
                else:
                    nc.vector.tensor_scalar_add(dst, dst, EPSP)

        if phases < 4:
            raise _PhaseCut
        # ---------------- denominator via K1 = causal @ kf ----------------
        recip = [P.tile([128, 12], F32, tag=f"recip{lt}", name=f"recip{lt}")
                 for lt in range(LT)]
        for i in range(LT):
            ka = pst([128, 384])
            kb = pst([128, 384])
            for j in range(i + 1):
                m = ones128 if j < i else maskd
                nc.tensor.matmul(ka, m, kf[j][:, 0:384],
                                 start=(j == 0), stop=(j == i))
                nc.tensor.matmul(kb, m, kf[j][:, 384:768],
                                 start=(j == 0), stop=(j == i))
            dn = dn_p.tile([128, H * F], F32, tag="dn", name="dn")
            nc.vector.tensor_mul(dn[:, 0:384], qf[i][:, 0:384], ka)
            nc.vector.tensor_mul(dn[:, 384:768], qf[i][:, 384:768], kb)
            den = sm_p.tile([128, 12], F32, tag="den", name="den")
            nc.vector.reduce_sum(den, dn.rearrange("p (h f) -> p h f", f=64),
                                 axis=mybir.AxisListType.X)
            nc.vector.tensor_scalar_add(den, den, EPS)
            nc.vector.reciprocal(recip[i], den)
            for h in range(H):
                nc.gpsimd.tensor_scalar_mul(qf[i][:, h * 64:(h + 1) * 64],
                                            qf[i][:, h * 64:(h + 1) * 64],
                                            recip[i][:, h:h + 1])

        if phases < 5:
            raise _PhaseCut
        # ---------------- transpose qf/kf -> [f, l] ----------------
        # qfT[t] paired: rows 0:64 = head 2t, rows 64:128 = head 2t+1.
        # kfTz[h] per head, other parity's rows zeroed, so the score matmul
        # runs full K=128 (zeros annihilate the other head in qfT).
        qfT = [P.tile([128, L], AD, tag=f"qfT{t}", name=f"qfT{t}")
               for t in range(NH2)]
        kfTz = [P.tile([128, L], AD, tag=f"cT{h}", name=f"kfTz{h}")
                for h in range(H)]
        for h in range(H):
            dead = slice((1 - h % 2) * 64, (1 - h % 2) * 64 + 64)
            nc.scalar.mul(kfTz[h][dead, :], b_inp_v[dead, 0:L], 0.0)
        for lt in range(LT):
            for t in range(NH2):
                for qk in range(2):
                    src = (qf if qk == 0 else kf)[lt][:, t * 128:(t + 1) * 128]
                    if attn_bf16:
                        p = pst([128, 128], BF16)
                        nc.tensor.transpose(p, src, identt)
                    elif qk == 0:
                        p = pst([128, 128], F32)
                        nc.tensor.transpose(p, src, ident)
                    else:
                        p = pst([128, 128], F32R)
                        nc.tensor.transpose(p, src, identt)
                    if qk == 0:
                        nc.vector.tensor_copy(qfT[t][:, lt * 128:(lt + 1) * 128], p)
                    else:
                        nc.vector.tensor_copy(
                            kfTz[2 * t][0:64, lt * 128:(lt + 1) * 128], p[0:64, :])
                        nc.vector.tensor_copy(
                            kfTz[2 * t + 1][64:128, lt * 128:(lt + 1) * 128],
                            p[64:128, :])

        if phases < 6:
            raise _PhaseCut
        # ---------------- scores ST[j,i] = kf @ qfT (causal) ----------------
        # ST_sb[h][j] covers i-columns [j*128, 512) ; diagonal block masked
        ST_sb = [[None] * LT for _ in range(H)]
        aT_all = [P.tile([128, L], AD, tag=f"aT{t}", name=f"aT{t}")
                  for t in range(NH2)]
        for t in range(NH2):
            pa = pst([128, L])
            for hh in range(2):
                h = 2 * t + hh
                for j in range(LT):
                    n = L - j * 128
                    pq = pst([128, n])
                    nc.tensor.matmul(
                        pq,
                        kfTz[h][:, j * 128:(j + 1) * 128],
                        qfT[t][:, j * 128:L],
                        start=True, stop=True)
                    st = st_p.tile([128, n], AD, tag="st", name="st")
                    nc.vector.tensor_mul(st[:, 0:128], pq[:, 0:128], maskd)
                    if n > 128:
                        nc.scalar.copy(st[:, 128:n], pq[:, 128:n])
                    ST_sb[h][j] = st
            for j in range(LT):
                for hh in range(2):
                    h = 2 * t + hh
                    nc.tensor.matmul(
                        pa[:, j * 128:L],
                        v_pboth[j][:, h * 128:(h + 1) * 128],
                        ST_sb[h][j],
                        start=(j == 0 and hh == 0),
                        stop=(j == LT - 1 and hh == 1))
            nc.vector.tensor_copy(aT_all[t], pa)

        if phases < 7:
            raise _PhaseCut
        # ---------------- output projection ----------------
        wo = wo_b if attn_bf16 else w_out_sb
        for lt in range(LT):
            po = [pst([128, 384]) for _ in range(2)]
            for et in range(ET):
                lhsT = aT_all[et][:, lt * 128:(lt + 1) * 128]
                for nh in range(2):
                    nc.tensor.matmul(po[nh], lhsT,
                                     wo[et][:, nh * 384:(nh + 1) * 384],
                                     start=(et == 0), stop=(et == ET - 1))
            osb = osb_p.tile([128, E], F32, tag="osb", name="osb")
            for nh in range(2):
                nc.vector.tensor_add(osb[:, nh * 384:(nh + 1) * 384], po[nh],
                                     b_out_sb[:, nh * 384:(nh + 1) * 384])
            nc.sync.dma_start(out=out_d[lt * 128:(lt + 1) * 128, :], in_=osb)
      except _PhaseCutE:
        pass

    if fix_waits:
        _fix_waits(nc)
    return nc


_CACHE = {}


def _get_nc():
    if "nc" not in _CACHE:
        _CACHE["nc"] = build_nc()
    return _CACHE["nc"]


def _host_consts(attn_bf16=ATTN_BF16):
    import ml_dtypes
    ad = ml_dtypes.bfloat16 if attn_bf16 else np.float32
    ident = np.eye(128, dtype=np.float32)
    return {
        "ident": ident,
        "ident_r": ident,
        "ident_a": ident.astype(ml_dtypes.bfloat16),
        "mask_diag": np.triu(np.ones((128, 128), dtype=np.float32)).astype(ad),
        "ones128": np.ones((128, 128), dtype=ad),
    }


def _in_maps(x, w_inp, b_inp, w_out, b_out, omega):
    f = lambda a: np.ascontiguousarray(np.asarray(a), dtype=np.float32)
    x, w_inp, b_inp = f(x), f(w_inp), f(b_inp)
    w_out, b_out, omega = f(w_out), f(b_out), f(omega)
    consts = _host_consts()
    maps = []
    for c in range(B):
        m = {"x": x[c], "w_inp": w_inp[0], "b_inp": b_inp,
             "w_out": w_out[0], "b_out": b_out, "omega": omega}
        m.update(consts)
        maps.append(m)
    return maps


def kernel(x, w_inp, b_inp, w_out, b_out, omega):
    nc = _get_nc()
    maps = _in_maps(x, w_inp, b_inp, w_out, b_out, omega)
    res = bass_utils.run_bass_kernel_spmd(nc, maps, core_ids=list(range(B)))
    return np.stack([res.results[c]["out"] for c in range(B)])


def run_traced(x, w_inp, b_inp, w_out, b_out, omega):
    """kernel() + HW time estimate. NTFF tracing is unavailable under this
    axon deployment, so time by wall-clock deltas on repeated dispatches."""
    import time
    from concourse import bass2jax
    nc = _get_nc()
    maps = _in_maps(x, w_inp, b_inp, w_out, b_out, omega)
    res = bass_utils.run_bass_kernel_spmd(nc, maps, core_ids=list(range(B)))
    out = np.stack([res.results[c]["out"] for c in range(B)])
    times = []
    for _ in range(6):
        t0 = time.perf_counter()
        bass2jax.run_bass_via_pjrt(nc, maps, n_cores=B)
        times.append(time.perf_counter() - t0)
    exec_ns = int(min(times) * 1e9)
    return out, exec_ns


# revision 22
# speedup vs baseline: 1.0520x; 1.0520x over previous
"""Trainium2 Bass kernel: FAVOR (Performer) causal linear attention block.

Per batch element (data-parallel over 8 NeuronCores):
  c = x @ w_inp + b_inp; q,k,v = split(c)
  qf/kf = rfm_softmax(q/k, omega)             (FAVOR random feature maps)
  a     = causal_linear_attention(qf, kf, v)  (masked score matmuls)
  out   = a @ w_out + b_out
"""

import numpy as np
from contextlib import ExitStack

import concourse.bass as bass
import concourse.tile as tile
from concourse import mybir
from concourse import bass_utils
import bass_rust

F32 = mybir.dt.float32
F32R = mybir.dt.float32r
BF16 = mybir.dt.bfloat16
AF = mybir.ActivationFunctionType

B, L, E, H, Dh, F = 8, 512, 768, 12, 64, 64
O3 = 3 * E
LT = L // 128      # 4 l-chunks
ET = E // 128      # 6 e-chunks
NH2 = H // 2       # 6 head pairs
EPS = 1e-6
LN8 = 2.0794415416798357   # 0.5 * ln(F)
SCALE_D = float(Dh) ** -0.25
EPSP = EPS * (float(F) ** -0.5)

ATTN_BF16 = False  # attention-path dtype switch


def _fix_waits(nc, cap=1):
    """Walrus codegen in this toolchain allows a single sync-wait per
    instruction; hoist excess waits onto injected same-engine NoOps placed
    directly before the offender (no reordering, deadlock-free)."""
    n = 0
    for fn in nc.m.functions:
        for bb in fn.blocks:
            insts = bb.instructions
            i = 0
            while i < len(insts):
                inst = insts[i]
                si = inst.sync_info
                if si is not None:
                    ow = list(si.on_wait)
                    if len(ow) > cap:
                        excess, keep = ow[:-cap], ow[-cap:]
                        si.on_wait = keep
                        for w in excess:
                            n += 1
                            nop = bass_rust.InstNoOp(
                                name=f"waitnop_{n}",
                                engine=inst.engine,
                                sync_info=bass_rust.SyncInfo(
                                    on_wait=[w], on_update=[]),
                            )
                            insts.insert(i, nop)
                            i += 1
                i += 1
    return n


def build_nc(attn_bf16=ATTN_BF16, fix_waits=True, phases=99):
    nc = bass.Bass("TRN2", target_bir_lowering=False, debug=False, num_devices=8)
    AD = BF16 if attn_bf16 else F32R   # attn-path matmul-operand dtype
    QD = BF16 if attn_bf16 else F32    # qf dtype
    KD = BF16 if attn_bf16 else F32R   # kf dtype (K1 matmul rhs)
    WD = F32 if attn_bf16 else F32R    # w_out DMA dtype

    x_d = nc.dram_tensor("x", [L, E], F32, kind="ExternalInput").ap()
    w_inp_d = nc.dram_tensor("w_inp", [E, O3], F32R, kind="ExternalInput").ap()
    b_inp_d = nc.dram_tensor("b_inp", [O3], F32, kind="ExternalInput").ap()
    w_out_d = nc.dram_tensor("w_out", [E, E], WD, kind="ExternalInput").ap()
    b_out_d = nc.dram_tensor("b_out", [E], F32, kind="ExternalInput").ap()
    omega_d = nc.dram_tensor("omega", [F, Dh], F32, kind="ExternalInput").ap()
    ident_d = nc.dram_tensor("ident", [128, 128], F32, kind="ExternalInput").ap()
    identr_d = nc.dram_tensor("ident_r", [128, 128], F32R, kind="ExternalInput").ap()
    identa_d = nc.dram_tensor("ident_a", [128, 128], BF16, kind="ExternalInput").ap()
    maskd_d = nc.dram_tensor("mask_diag", [128, 128], AD, kind="ExternalInput").ap()
    ones_d = nc.dram_tensor("ones128", [128, 128], AD, kind="ExternalInput").ap()
    out_d = nc.dram_tensor("out", [L, E], F32, kind="ExternalOutput").ap()

    def bc(ap, p=128):
        # broadcast a 1-D DRAM AP across p partitions
        return bass.AP(tensor=ap.tensor, offset=ap.offset,
                       ap=[[0, p]] + [list(d) for d in ap.ap])

    class _PhaseCutE(Exception):
        pass
    global _PhaseCut
    _PhaseCut = _PhaseCutE
    with tile.TileContext(nc) as tc, ExitStack() as ctx:
      try:
        P = ctx.enter_context(tc.tile_pool(name="persist", bufs=1))
        wqk_p = ctx.enter_context(tc.tile_pool(name="wqk", bufs=3))
        wv_p = ctx.enter_context(tc.tile_pool(name="wv", bufs=4))
        xin_p = ctx.enter_context(tc.tile_pool(name="xin", bufs=1))
        ssub_p = ctx.enter_context(tc.tile_pool(name="ssub", bufs=3))
        bexp_p = ctx.enter_context(tc.tile_pool(name="bexp", bufs=2))
        st_p = ctx.enter_context(tc.tile_pool(name="stp", bufs=6))
        dn_p = ctx.enter_context(tc.tile_pool(name="dnp", bufs=2))
        sm_p = ctx.enter_context(tc.tile_pool(name="smp", bufs=10))
        osb_p = ctx.enter_context(tc.tile_pool(name="osb", bufs=2))
        ps = ctx.enter_context(tc.tile_pool(name="ps", bufs=8, space="PSUM"))

        cnt = [0]

        def pst(shape, dtype=F32):
            cnt[0] += 1
            return ps.tile(shape, dtype, tag="ps", name=f"pst{cnt[0]}")

        # ---------------- x load (transposes interleaved with QKV) --------
        xT = [P.tile([128, L], F32R, tag=f"xT{et}", name=f"xT{et}")
              for et in range(ET)]
        xins = []
        for lt in range(LT):
            xin = xin_p.tile([128, E], F32, tag=f"xin{lt}", name=f"xin{lt}")
            nc.gpsimd.dma_start(out=xin, in_=x_d[lt * 128:(lt + 1) * 128, :])
            xins.append(xin)
        # ---------------- constants ----------------
        ident = P.tile([128, 128], F32, tag="ident", name="ident")
        nc.gpsimd.dma_start(out=ident, in_=ident_d)
        omega_w = P.tile([128, 128], F32, tag="omega_w", name="omega_w")
        for rr_ in range(2):
            for cc_ in range(2):
                nc.gpsimd.dma_start(
                    out=omega_w[rr_ * 64:(rr_ + 1) * 64, cc_ * 64:(cc_ + 1) * 64],
                    in_=omega_d)
        identt = P.tile([128, 128], BF16 if attn_bf16 else F32R,
                        tag="identt", name="identt")
        nc.gpsimd.dma_start(out=identt, in_=identa_d if attn_bf16 else identr_d)
        maskd = P.tile([128, 128], AD, tag="maskd", name="maskd")
        nc.gpsimd.dma_start(out=maskd, in_=maskd_d)
        ones128 = P.tile([128, 128], AD, tag="ones128", name="ones128")
        nc.gpsimd.dma_start(out=ones128, in_=ones_d)

        b_inpT = P.tile([128, 12], F32, tag="b_inpT", name="b_inpT")
        nc.gpsimd.dma_start(out=b_inpT,
                          in_=b_inp_d.rearrange("(j p) -> p j", p=128)[:, 0:12])
        b_inp_v = P.tile([128, E], F32, tag="b_inp_v", name="b_inp_v")
        nc.gpsimd.dma_start(out=b_inp_v, in_=bc(b_inp_d[2 * E:3 * E]))
        b_out_sb = P.tile([128, E], F32, tag="b_out_sb", name="b_out_sb")
        nc.gpsimd.dma_start(out=b_out_sb, in_=bc(b_out_d))


        # w_out resident (reused by all 4 l-chunks)
        w_out_sb = []
        for et in range(ET):
            t = P.tile([128, E], WD, tag=f"wo{et}", name=f"wo{et}")
            nc.sync.dma_start(out=t, in_=w_out_d[et * 128:(et + 1) * 128, :])
            w_out_sb.append(t)
        if attn_bf16:
            wo_b = []
            for et in range(ET):
                t = P.tile([128, E], BF16, tag=f"wob{et}", name=f"wob{et}")
                nc.vector.tensor_copy(t, w_out_sb[et])
                wo_b.append(t)


        if phases < 1:
            raise _PhaseCut
        for et in range(ET):
            for lt in range(LT):
                p = pst([128, 128])
                nc.tensor.transpose(p, xins[lt][:, et * 128:(et + 1) * 128],
                                    ident)
                if lt % 2 == 0:
                    nc.vector.tensor_copy(xT[et][:, lt * 128:(lt + 1) * 128], p)
                else:
                    nc.scalar.copy(xT[et][:, lt * 128:(lt + 1) * 128], p)

        # ---------------- QKV: q,k transposed ----------------
        # cT[ot] [o=128, l=512]; ot 0..5 -> q channels, 6..11 -> k channels
        cT = [P.tile([128, L], F32R, tag=f"cT{ot}", name=f"cT{ot}")
              for ot in range(12)]
        for grp in range(2):  # 0: q section, 1: k section
            pcs = [pst([128, L]) for _ in range(6)]
            for et in range(ET):
                wt = wqk_p.tile([128, E], F32R, tag="wqk", name="wqk")
                nc.sync.dma_start(
                    out=wt,
                    in_=w_inp_d[et * 128:(et + 1) * 128, grp * E:(grp + 1) * E])
                for o in range(6):
                    nc.tensor.matmul(pcs[o], wt[:, o * 128:(o + 1) * 128],
                                     xT[et], start=(et == 0), stop=(et == ET - 1))
            for o in range(6):
                ot = grp * 6 + o
                nc.scalar.activation(cT[ot], pcs[o], AF.Identity,
                                     bias=b_inpT[:, ot:ot + 1], scale=1.0)

        if phases < 2:
            raise _PhaseCut
        # ---------------- QKV: v natural [l, o] ----------------
        # v stored zero-padded per head: head h lives in cols
        # [h*128 + (h%2)*64, +64) of v_pboth, rest zero -> every attn matmul
        # runs with a full [128,128] lhsT (no PE array-tiling modes)
        v_pboth = [P.tile([128, H * 128], AD, tag=f"vp{lt}", name=f"vp{lt}")
                   for lt in range(LT)]
        for lt in range(LT):
            nc.scalar.mul(v_pboth[lt][:, 0:E], b_inp_v, 0.0)
            nc.scalar.mul(v_pboth[lt][:, E:2 * E], b_inp_v, 0.0)
        for nh in range(2):
            pv = [pst([128, 384]) for _ in range(LT)]
            for et in range(ET):
                wt = wv_p.tile([128, 384], F32R, tag="wv", name="wv")
                nc.sync.dma_start(
                    out=wt,
                    in_=w_inp_d[et * 128:(et + 1) * 128,
                                2 * E + nh * 384:2 * E + (nh + 1) * 384])
                for lt in range(LT):
                    nc.tensor.matmul(pv[lt], xT[et][:, lt * 128:(lt + 1) * 128],
                                     wt, start=(et == 0), stop=(et == ET - 1))
            for lt in range(LT):
                pvr = pv[lt].rearrange("p (t x) -> p t x", x=128)
                bvr = b_inp_v[:, nh * 384:(nh + 1) * 384].rearrange(
                    "p (t x) -> p t x", x=128)
                vpr = v_pboth[lt].rearrange("p (t x) -> p t x", x=256)[
                    :, nh * 3:(nh + 1) * 3, :]
                # even heads of this half -> block offset 0; odd -> offset 192
                nc.vector.tensor_add(vpr[:, :, 0:64], pvr[:, :, 0:64],
                                     bvr[:, :, 0:64])
                nc.vector.tensor_add(vpr[:, :, 192:256], pvr[:, :, 64:128],
                                     bvr[:, :, 64:128])

        if phases < 3:
            raise _PhaseCut
        pt = pst([128, 128])
        nc.tensor.transpose(pt, omega_w, ident)
        oz = []  # oz[0]: rows 0:64 live; oz[1]: rows 64:128 live
        for par in range(2):
            t = P.tile([128, 64], F32R, tag=f"oz{par}", name=f"oz{par}")
            nc.scalar.mul(t, b_inp_v[:, 0:64], 0.0)
            half = slice(par * 64, par * 64 + 64)
            nc.scalar.mul(t[half, :], pt[half, 0:64], SCALE_D)
            oz.append(t)
        # ---------------- FAVOR feature maps ----------------
        qf = [P.tile([128, H * F], QD, tag=f"qf{lt}", name=f"qf{lt}")
              for lt in range(LT)]
        kf = [P.tile([128, H * F], KD, tag=f"kf{lt}", name=f"kf{lt}")
              for lt in range(LT)]
        for qk in (1, 0):  # k first: K1 can start while q maps compute
            for lt in range(LT):
                sA = pst([128, 512])
                sB = pst([128, 256])
                for h in range(H):
                    lhsT = cT[qk * 6 + h // 2][:, lt * 128:(lt + 1) * 128]
                    rhs = oz[h % 2]
                    dst = (sA[:, (h % 8) * 64:(h % 8) * 64 + 64] if h < 8
                           else sB[:, (h - 8) * 64:(h - 8) * 64 + 64])
                    nc.tensor.matmul(dst, lhsT, rhs, start=True, stop=True)
                d_all = sm_p.tile([128, 12], F32, tag="d_all", name="d_all")
                m_all = sm_p.tile([128, 12], F32, tag="m_all", name="m_all")
                nc.vector.reduce_sum(d_all[:, 0:8],
                                     sA.rearrange("p (h f) -> p h f", f=64),
                                     axis=mybir.AxisListType.X)
                nc.vector.reduce_sum(d_all[:, 8:12],
                                     sB.rearrange("p (h f) -> p h f", f=64),
                                     axis=mybir.AxisListType.X)
                nc.vector.reduce_max(m_all[:, 0:8],
                                     sA.rearrange("p (h f) -> p h f", f=64),
                                     axis=mybir.AxisListType.X)
                nc.vector.reduce_max(m_all[:, 8:12],
                                     sB.rearrange("p (h f) -> p h f", f=64),
                                     axis=mybir.AxisListType.X)
                bias_all = sm_p.tile([128, 12], F32, tag="bias_all",
                                     name="bias_all")
                nc.vector.tensor_scalar(bias_all, d_all, -0.5, -LN8,
                                        op0=mybir.AluOpType.mult,
                                        op1=mybir.AluOpType.add)
                if qk == 0:
                    nc.vector.tensor_sub(bias_all, bias_all, m_all)
                else:
                    mk = sm_p.tile([128, 1], F32, tag="mk", name="mk")
                    nc.vector.reduce_max(mk, m_all, axis=mybir.AxisListType.X)
                    nc.vector.tensor_sub(bias_all, bias_all,
                                         mk.to_broadcast((128, 12)))
                bias_exp = bexp_p.tile([128, 12, 64], F32, tag="bexp",
                                       name="bexp")
                nc.gpsimd.tensor_copy(
                    bias_exp, bias_all.unsqueeze(2).broadcast_to((128, 12, 64)))
                s_sub = ssub_p.tile([128, H * F], F32, tag="ssub", name="ssub")
                nc.vector.tensor_add(s_sub[:, 0:512], sA, bias_exp[:, 0:8, :])
                nc.vector.tensor_add(s_sub[:, 512:768], sB, bias_exp[:, 8:12, :])
                dst = qf[lt] if qk == 0 else kf[lt]
                nc.scalar.activation(dst, s_sub, AF.Exp)
                nc.gpsimd.tensor_scalar_add(dst, dst, EPSP)

        if phases < 4:
            raise _PhaseCut
        # kf -> [f, l] per head, zero-padded (other parity rows = 0) so the
        # score matmul runs full K=128; reuses k-section cT slots (freed first)
        kfTz = [P.tile([128, L], AD, tag=f"cT{(h + 6) % 12}", name=f"kfTz{h}")
                for h in range(H)]
        for h in range(H):
            dead = slice((1 - h % 2) * 64, (1 - h % 2) * 64 + 64)
            nc.scalar.mul(kfTz[h][dead, :], b_inp_v[dead, 0:L], 0.0)
        for lt in range(LT):
            for t in range(NH2):
                if attn_bf16:
                    p = pst([128, 128], BF16)
                    nc.tensor.transpose(p, kf[lt][:, t * 128:(t + 1) * 128],
                                        identt)
                else:
                    p = pst([128, 128], F32R)
                    nc.tensor.transpose(p, kf[lt][:, t * 128:(t + 1) * 128],
                                        identt)
                nc.vector.tensor_copy(
                    kfTz[2 * t][0:64, lt * 128:(lt + 1) * 128], p[0:64, :])
                nc.vector.tensor_copy(
                    kfTz[2 * t + 1][64:128, lt * 128:(lt + 1) * 128],
                    p[64:128, :])

        # ---------------- denominator via K1 = causal @ kf ----------------
        recip = [P.tile([128, 12], F32, tag=f"recip{lt}", name=f"recip{lt}")
                 for lt in range(LT)]
        for i in range(LT):
            ka = pst([128, 384])
            kb = pst([128, 384])
            for j in range(i + 1):
                m = ones128 if j < i else maskd
                nc.tensor.matmul(ka, m, kf[j][:, 0:384],
                                 start=(j == 0), stop=(j == i))
                nc.tensor.matmul(kb, m, kf[j][:, 384:768],
                                 start=(j == 0), stop=(j == i))
            dn = dn_p.tile([128, H * F], F32, tag="dn", name="dn")
            nc.vector.tensor_mul(dn[:, 0:384], qf[i][:, 0:384], ka)
            nc.vector.tensor_mul(dn[:, 384:768], qf[i][:, 384:768], kb)
            den = sm_p.tile([128, 12], F32, tag="den", name="den")
            nc.vector.reduce_sum(den, dn.rearrange("p (h f) -> p h f", f=64),
                                 axis=mybir.AxisListType.X)
            nc.vector.tensor_scalar_add(den, den, EPS)
            nc.vector.reciprocal(recip[i], den)
            for h in range(H):
                nc.gpsimd.tensor_scalar_mul(qf[i][:, h * 64:(h + 1) * 64],
                                            qf[i][:, h * 64:(h + 1) * 64],
                                            recip[i][:, h:h + 1])

        if phases < 5:
            raise _PhaseCut
        # ---------------- transpose qf -> [f, l] pairs ----------------
        # qfT[t] paired: rows 0:64 = head 2t, rows 64:128 = head 2t+1
        qfT = [P.tile([128, L], AD, tag=f"qfT{t}", name=f"qfT{t}")
               for t in range(NH2)]
        for lt in range(LT):
            for t in range(NH2):
                if attn_bf16:
                    p = pst([128, 128], BF16)
                    nc.tensor.transpose(p, qf[lt][:, t * 128:(t + 1) * 128],
                                        identt)
                else:
                    p = pst([128, 128], F32)
                    nc.tensor.transpose(p, qf[lt][:, t * 128:(t + 1) * 128],
                                        ident)
                nc.vector.tensor_copy(qfT[t][:, lt * 128:(lt + 1) * 128], p)

        if phases < 6:
            raise _PhaseCut
        # ---------------- scores ST[j,i] = kf @ qfT (causal) ----------------
        # ST_sb[h][j] covers i-columns [j*128, 512) ; diagonal block masked
        ST_sb = [[None] * LT for _ in range(H)]
        aT_all = [P.tile([128, L], AD, tag=f"aT{t}", name=f"aT{t}")
                  for t in range(NH2)]
        for t in range(NH2):
            pa = pst([128, L])
            for hh in range(2):
                h = 2 * t + hh
                for j in range(LT):
                    n = L - j * 128
                    pq = pst([128, n])
                    nc.tensor.matmul(
                        pq,
                        kfTz[h][:, j * 128:(j + 1) * 128],
                        qfT[t][:, j * 128:L],
                        start=True, stop=True)
                    st = st_p.tile([128, n], AD, tag="st", name="st")
                    nc.vector.tensor_mul(st[:, 0:128], pq[:, 0:128], maskd)
                    if n > 128:
                        nc.scalar.copy(st[:, 128:n], pq[:, 128:n])
                    ST_sb[h][j] = st
            for j in range(LT):
                for hh in range(2):
                    h = 2 * t + hh
                    nc.tensor.matmul(
                        pa[:, j * 128:L],
                        v_pboth[j][:, h * 128:(h + 1) * 128],
                        ST_sb[h][j],
                        start=(j == 0 and hh == 0),
                        stop=(j == LT - 1 and hh == 1))
            nc.vector.tensor_copy(aT_all[t], pa)

        if phases < 7:
            raise _PhaseCut
        # ---------------- output projection ----------------
        wo = wo_b if attn_bf16 else w_out_sb
        for lt in range(LT):
            po = [pst([128, 384]) for _ in range(2)]
            for et in range(ET):
                lhsT = aT_all[et][:, lt * 128:(lt + 1) * 128]
                for nh in range(2):
                    nc.tensor.matmul(po[nh], lhsT,
                                     wo[et][:, nh * 384:(nh + 1) * 384],
                                     start=(et == 0), stop=(et == ET - 1))
            osb = osb_p.tile([128, E], F32, tag="osb", name="osb")
            for nh in range(2):
                nc.vector.tensor_add(osb[:, nh * 384:(nh + 1) * 384], po[nh],
                                     b_out_sb[:, nh * 384:(nh + 1) * 384])
            nc.sync.dma_start(out=out_d[lt * 128:(lt + 1) * 128, :], in_=osb)
      except _PhaseCutE:
        pass

    if fix_waits:
        _fix_waits(nc)
    return nc


_CACHE = {}


def _get_nc():
    if "nc" not in _CACHE:
        _CACHE["nc"] = build_nc()
    return _CACHE["nc"]


def _host_consts(attn_bf16=ATTN_BF16):
    import ml_dtypes
    ad = ml_dtypes.bfloat16 if attn_bf16 else np.float32
    ident = np.eye(128, dtype=np.float32)
    return {
        "ident": ident,
        "ident_r": ident,
        "ident_a": ident.astype(ml_dtypes.bfloat16),
        "mask_diag": np.triu(np.ones((128, 128), dtype=np.float32)).astype(ad),
        "ones128": np.ones((128, 128), dtype=ad),
    }


def _in_maps(x, w_inp, b_inp, w_out, b_out, omega):
    f = lambda a: np.ascontiguousarray(np.asarray(a), dtype=np.float32)
    x, w_inp, b_inp = f(x), f(w_inp), f(b_inp)
    w_out, b_out, omega = f(w_out), f(b_out), f(omega)
    consts = _host_consts()
    maps = []
    for c in range(B):
        m = {"x": x[c], "w_inp": w_inp[0], "b_inp": b_inp,
             "w_out": w_out[0], "b_out": b_out, "omega": omega}
        m.update(consts)
        maps.append(m)
    return maps


def kernel(x, w_inp, b_inp, w_out, b_out, omega):
    nc = _get_nc()
    maps = _in_maps(x, w_inp, b_inp, w_out, b_out, omega)
    res = bass_utils.run_bass_kernel_spmd(nc, maps, core_ids=list(range(B)))
    return np.stack([res.results[c]["out"] for c in range(B)])


def run_traced(x, w_inp, b_inp, w_out, b_out, omega):
    """kernel() + HW time estimate. NTFF tracing is unavailable under this
    axon deployment, so time by wall-clock deltas on repeated dispatches."""
    import time
    from concourse import bass2jax
    nc = _get_nc()
    maps = _in_maps(x, w_inp, b_inp, w_out, b_out, omega)
    res = bass_utils.run_bass_kernel_spmd(nc, maps, core_ids=list(range(B)))
    out = np.stack([res.results[c]["out"] for c in range(B)])
    times = []
    for _ in range(6):
        t0 = time.perf_counter()
        bass2jax.run_bass_via_pjrt(nc, maps, n_cores=B)
        times.append(time.perf_counter() - t0)
    exec_ns = int(min(times) * 1e9)
    return out, exec_ns


# revision 26
# speedup vs baseline: 1.1249x; 1.0693x over previous
"""Trainium2 Bass kernel: FAVOR (Performer) causal linear attention block.

Per batch element (data-parallel over 8 NeuronCores):
  c = x @ w_inp + b_inp; q,k,v = split(c)
  qf/kf = rfm_softmax(q/k, omega)             (FAVOR random feature maps)
  a     = causal_linear_attention(qf, kf, v)  (masked score matmuls)
  out   = a @ w_out + b_out
"""

import numpy as np
from contextlib import ExitStack

import concourse.bass as bass
import concourse.tile as tile
from concourse import mybir
from concourse import bass_utils
import bass_rust

F32 = mybir.dt.float32
F32R = mybir.dt.float32r
BF16 = mybir.dt.bfloat16
AF = mybir.ActivationFunctionType

B, L, E, H, Dh, F = 8, 512, 768, 12, 64, 64
O3 = 3 * E
LT = L // 128      # 4 l-chunks
ET = E // 128      # 6 e-chunks
NH2 = H // 2       # 6 head pairs
EPS = 1e-6
LN8 = 2.0794415416798357   # 0.5 * ln(F)
SCALE_D = float(Dh) ** -0.25
EPSP = EPS * (float(F) ** -0.5)

ATTN_BF16 = False  # attention-path dtype switch


def _fix_waits(nc, cap=1):
    """Walrus codegen in this toolchain allows a single sync-wait per
    instruction; hoist excess waits onto injected same-engine NoOps placed
    directly before the offender (no reordering, deadlock-free)."""
    n = 0
    for fn in nc.m.functions:
        for bb in fn.blocks:
            insts = bb.instructions
            i = 0
            while i < len(insts):
                inst = insts[i]
                si = inst.sync_info
                if si is not None:
                    ow = list(si.on_wait)
                    if len(ow) > cap:
                        excess, keep = ow[:-cap], ow[-cap:]
                        si.on_wait = keep
                        for w in excess:
                            n += 1
                            nop = bass_rust.InstNoOp(
                                name=f"waitnop_{n}",
                                engine=inst.engine,
                                sync_info=bass_rust.SyncInfo(
                                    on_wait=[w], on_update=[]),
                            )
                            insts.insert(i, nop)
                            i += 1
                i += 1
    return n


def build_nc(attn_bf16=ATTN_BF16, fix_waits=True, phases=99):
    nc = bass.Bass("TRN2", target_bir_lowering=False, debug=False, num_devices=8)
    AD = BF16 if attn_bf16 else F32R   # attn-path matmul-operand dtype
    QD = BF16 if attn_bf16 else F32    # qf dtype
    KD = BF16 if attn_bf16 else F32R   # kf dtype (K1 matmul rhs)
    WD = F32 if attn_bf16 else F32R    # w_out DMA dtype

    x_d = nc.dram_tensor("x", [L, E], F32, kind="ExternalInput").ap()
    w_inp_d = nc.dram_tensor("w_inp", [E, O3], F32R, kind="ExternalInput").ap()
    b_inp_d = nc.dram_tensor("b_inp", [O3], F32, kind="ExternalInput").ap()
    w_out_d = nc.dram_tensor("w_out", [E, E], WD, kind="ExternalInput").ap()
    b_out_d = nc.dram_tensor("b_out", [E], F32, kind="ExternalInput").ap()
    omega_d = nc.dram_tensor("omega", [F, Dh], F32, kind="ExternalInput").ap()
    ident_d = nc.dram_tensor("ident", [128, 128], F32, kind="ExternalInput").ap()
    identr_d = nc.dram_tensor("ident_r", [128, 128], F32R, kind="ExternalInput").ap()
    identa_d = nc.dram_tensor("ident_a", [128, 128], BF16, kind="ExternalInput").ap()
    maskd_d = nc.dram_tensor("mask_diag", [128, 128], AD, kind="ExternalInput").ap()
    ones_d = nc.dram_tensor("ones128", [128, 128], AD, kind="ExternalInput").ap()
    out_d = nc.dram_tensor("out", [L, E], F32, kind="ExternalOutput").ap()

    def bc(ap, p=128):
        # broadcast a 1-D DRAM AP across p partitions
        return bass.AP(tensor=ap.tensor, offset=ap.offset,
                       ap=[[0, p]] + [list(d) for d in ap.ap])

    class _PhaseCutE(Exception):
        pass
    global _PhaseCut
    _PhaseCut = _PhaseCutE
    with tile.TileContext(nc) as tc, ExitStack() as ctx:
      try:
        P = ctx.enter_context(tc.tile_pool(name="persist", bufs=1))
        wqk_p = ctx.enter_context(tc.tile_pool(name="wqk", bufs=3))
        wv_p = ctx.enter_context(tc.tile_pool(name="wv", bufs=4))
        xin_p = ctx.enter_context(tc.tile_pool(name="xin", bufs=1))
        ssub_p = ctx.enter_context(tc.tile_pool(name="ssub", bufs=3))
        bexp_p = ctx.enter_context(tc.tile_pool(name="bexp", bufs=2))
        st_p = ctx.enter_context(tc.tile_pool(name="stp", bufs=6))
        dn_p = ctx.enter_context(tc.tile_pool(name="dnp", bufs=2))
        sm_p = ctx.enter_context(tc.tile_pool(name="smp", bufs=10))
        osb_p = ctx.enter_context(tc.tile_pool(name="osb", bufs=2))
        ps = ctx.enter_context(tc.tile_pool(name="ps", bufs=8, space="PSUM"))

        cnt = [0]

        def pst(shape, dtype=F32):
            cnt[0] += 1
            return ps.tile(shape, dtype, tag="ps", name=f"pst{cnt[0]}")

        # ---------------- x load (transposes interleaved with QKV) --------
        xT = [P.tile([128, L], F32R, tag=f"xT{et}", name=f"xT{et}")
              for et in range(ET)]
        xins = []
        for lt in range(LT):
            xin = xin_p.tile([128, E], F32, tag=f"xin{lt}", name=f"xin{lt}")
            nc.gpsimd.dma_start(out=xin, in_=x_d[lt * 128:(lt + 1) * 128, :])
            xins.append(xin)
        # ---------------- constants ----------------
        ident = P.tile([128, 128], F32, tag="ident", name="ident")
        nc.gpsimd.dma_start(out=ident, in_=ident_d)
        omega_w = P.tile([128, 128], F32, tag="omega_w", name="omega_w")
        for rr_ in range(2):
            for cc_ in range(2):
                nc.gpsimd.dma_start(
                    out=omega_w[rr_ * 64:(rr_ + 1) * 64, cc_ * 64:(cc_ + 1) * 64],
                    in_=omega_d)
        identt = P.tile([128, 128], BF16 if attn_bf16 else F32R,
                        tag="identt", name="identt")
        nc.gpsimd.dma_start(out=identt, in_=identa_d if attn_bf16 else identr_d)
        maskd = P.tile([128, 128], AD, tag="maskd", name="maskd")
        nc.gpsimd.dma_start(out=maskd, in_=maskd_d)
        ones128 = P.tile([128, 128], AD, tag="ones128", name="ones128")
        nc.gpsimd.dma_start(out=ones128, in_=ones_d)

        b_inpT = P.tile([128, 12], F32, tag="b_inpT", name="b_inpT")
        nc.gpsimd.dma_start(out=b_inpT,
                          in_=b_inp_d.rearrange("(j p) -> p j", p=128)[:, 0:12])
        b_inp_v = P.tile([128, E], F32, tag="b_inp_v", name="b_inp_v")
        nc.gpsimd.dma_start(out=b_inp_v, in_=bc(b_inp_d[2 * E:3 * E]))
        b_out_sb = P.tile([128, E], F32, tag="b_out_sb", name="b_out_sb")
        nc.gpsimd.dma_start(out=b_out_sb, in_=bc(b_out_d))


        # w_out resident (reused by all 4 l-chunks)
        w_out_sb = []
        for et in range(ET):
            t = P.tile([128, E], WD, tag=f"wo{et}", name=f"wo{et}")
            nc.sync.dma_start(out=t, in_=w_out_d[et * 128:(et + 1) * 128, :])
            w_out_sb.append(t)
        if attn_bf16:
            wo_b = []
            for et in range(ET):
                t = P.tile([128, E], BF16, tag=f"wob{et}", name=f"wob{et}")
                nc.vector.tensor_copy(t, w_out_sb[et])
                wo_b.append(t)


        if phases < 1:
            raise _PhaseCut
        for et in range(ET):
            for lt in range(LT):
                p = pst([128, 128])
                nc.tensor.transpose(p, xins[lt][:, et * 128:(et + 1) * 128],
                                    ident)
                if lt % 2 == 0:
                    nc.vector.tensor_copy(xT[et][:, lt * 128:(lt + 1) * 128], p)
                else:
                    nc.scalar.copy(xT[et][:, lt * 128:(lt + 1) * 128], p)

        # ---------------- QKV: q,k transposed ----------------
        # cT[ot] [o=128, l=512]; ot 0..5 -> q channels, 6..11 -> k channels
        cT = [P.tile([128, L], F32R, tag=f"cT{ot}", name=f"cT{ot}")
              for ot in range(12)]
        for grp in range(2):  # 0: q section, 1: k section
            pcs = [pst([128, L]) for _ in range(6)]
            for et in range(ET):
                wt = wqk_p.tile([128, E], F32R, tag="wqk", name="wqk")
                nc.sync.dma_start(
                    out=wt,
                    in_=w_inp_d[et * 128:(et + 1) * 128, grp * E:(grp + 1) * E])
                for o in range(6):
                    nc.tensor.matmul(pcs[o], wt[:, o * 128:(o + 1) * 128],
                                     xT[et], start=(et == 0), stop=(et == ET - 1))
            for o in range(6):
                ot = grp * 6 + o
                nc.scalar.activation(cT[ot], pcs[o], AF.Identity,
                                     bias=b_inpT[:, ot:ot + 1], scale=1.0)

        if phases < 2:
            raise _PhaseCut
        # ---------------- QKV: v natural [l, o] ----------------
        # v stored zero-padded per head: head h lives in cols
        # [h*128 + (h%2)*64, +64) of v_pboth, rest zero -> every attn matmul
        # runs with a full [128,128] lhsT (no PE array-tiling modes)
        v_pboth = [P.tile([128, H * 128], AD, tag=f"vp{lt}", name=f"vp{lt}")
                   for lt in range(LT)]
        for lt in range(LT):
            nc.scalar.mul(v_pboth[lt][:, 0:E], b_inp_v, 0.0)
            nc.scalar.mul(v_pboth[lt][:, E:2 * E], b_inp_v, 0.0)
        for nh in range(2):
            pv = [pst([128, 384]) for _ in range(LT)]
            for et in range(ET):
                wt = wv_p.tile([128, 384], F32R, tag="wv", name="wv")
                nc.sync.dma_start(
                    out=wt,
                    in_=w_inp_d[et * 128:(et + 1) * 128,
                                2 * E + nh * 384:2 * E + (nh + 1) * 384])
                for lt in range(LT):
                    nc.tensor.matmul(pv[lt], xT[et][:, lt * 128:(lt + 1) * 128],
                                     wt, start=(et == 0), stop=(et == ET - 1))
            for lt in range(LT):
                pvr = pv[lt].rearrange("p (t x) -> p t x", x=128)
                bvr = b_inp_v[:, nh * 384:(nh + 1) * 384].rearrange(
                    "p (t x) -> p t x", x=128)
                vpr = v_pboth[lt].rearrange("p (t x) -> p t x", x=256)[
                    :, nh * 3:(nh + 1) * 3, :]
                # even heads of this half -> block offset 0; odd -> offset 192
                nc.vector.tensor_add(vpr[:, :, 0:64], pvr[:, :, 0:64],
                                     bvr[:, :, 0:64])
                nc.vector.tensor_add(vpr[:, :, 192:256], pvr[:, :, 64:128],
                                     bvr[:, :, 64:128])

        if phases < 3:
            raise _PhaseCut
        pt = pst([128, 128])
        nc.tensor.transpose(pt, omega_w, ident)
        oz = []  # oz[0]: rows 0:64 live; oz[1]: rows 64:128 live
        for par in range(2):
            t = P.tile([128, 64], F32R, tag=f"oz{par}", name=f"oz{par}")
            nc.scalar.mul(t, b_inp_v[:, 0:64], 0.0)
            half = slice(par * 64, par * 64 + 64)
            nc.scalar.mul(t[half, :], pt[half, 0:64], SCALE_D)
            oz.append(t)
        # rowsums of oz -> diag comes from a tiny PE matmul instead of DVE
        wd2 = P.tile([128, 2], F32R, tag="wd2", name="wd2")
        with nc.allow_low_precision(reason="64-elt rowsum; f32r round ~1e-4"):
            nc.vector.reduce_sum(wd2[:, 0:1], oz[0], axis=mybir.AxisListType.X)
            nc.vector.reduce_sum(wd2[:, 1:2], oz[1], axis=mybir.AxisListType.X)
        # ---------------- FAVOR feature maps ----------------
        qf = [P.tile([128, H * F], QD, tag=f"qf{lt}", name=f"qf{lt}")
              for lt in range(LT)]
        kf = [P.tile([128, H * F], KD, tag=f"kf{lt}", name=f"kf{lt}")
              for lt in range(LT)]
        for qk in (1, 0):  # k first: K1 can start while q maps compute
            for lt in range(LT):
                sA = pst([128, 512])
                sB = pst([128, 256])
                pd = pst([128, 12])
                for o in range(6):
                    nc.tensor.matmul(pd[:, 2 * o:2 * o + 2],
                                     cT[qk * 6 + o][:, lt * 128:(lt + 1) * 128],
                                     wd2, start=True, stop=True)
                for h in range(H):
                    lhsT = cT[qk * 6 + h // 2][:, lt * 128:(lt + 1) * 128]
                    rhs = oz[h % 2]
                    dst = (sA[:, (h % 8) * 64:(h % 8) * 64 + 64] if h < 8
                           else sB[:, (h - 8) * 64:(h - 8) * 64 + 64])
                    nc.tensor.matmul(dst, lhsT, rhs, start=True, stop=True)
                m_all = sm_p.tile([128, 12], F32, tag="m_all", name="m_all")
                nc.vector.reduce_max(m_all[:, 0:8],
                                     sA.rearrange("p (h f) -> p h f", f=64),
                                     axis=mybir.AxisListType.X)
                nc.vector.reduce_max(m_all[:, 8:12],
                                     sB.rearrange("p (h f) -> p h f", f=64),
                                     axis=mybir.AxisListType.X)
                bias_all = sm_p.tile([128, 12], F32, tag="bias_all",
                                     name="bias_all")
                nc.vector.tensor_scalar(bias_all, pd, -0.5, -LN8,
                                        op0=mybir.AluOpType.mult,
                                        op1=mybir.AluOpType.add)
                if qk == 0:
                    nc.vector.tensor_sub(bias_all, bias_all, m_all)
                else:
                    mk = sm_p.tile([128, 1], F32, tag="mk", name="mk")
                    nc.vector.reduce_max(mk, m_all, axis=mybir.AxisListType.X)
                    nc.vector.tensor_sub(bias_all, bias_all,
                                         mk.to_broadcast((128, 12)))
                bias_exp = bexp_p.tile([128, 12, 64], F32, tag="bexp",
                                       name="bexp")
                nc.gpsimd.tensor_copy(
                    bias_exp, bias_all.unsqueeze(2).broadcast_to((128, 12, 64)))
                s_sub = ssub_p.tile([128, H * F], F32, tag="ssub", name="ssub")
                nc.vector.tensor_add(s_sub[:, 0:512], sA, bias_exp[:, 0:8, :])
                nc.vector.tensor_add(s_sub[:, 512:768], sB, bias_exp[:, 8:12, :])
                dst = qf[lt] if qk == 0 else kf[lt]
                nc.scalar.activation(dst, s_sub, AF.Exp)
                nc.gpsimd.tensor_scalar_add(dst, dst, EPSP)

        if phases < 4:
            raise _PhaseCut
        # kf -> [f, l] per head, zero-padded (other parity rows = 0) so the
        # score matmul runs full K=128; reuses k-section cT slots (freed first)
        kfTz = [P.tile([128, L], AD, tag=f"cT{(h + 6) % 12}", name=f"kfTz{h}")
                for h in range(H)]
        for h in range(H):
            dead = slice((1 - h % 2) * 64, (1 - h % 2) * 64 + 64)
            nc.scalar.mul(kfTz[h][dead, :], b_inp_v[dead, 0:L], 0.0)
        for lt in range(LT):
            for t in range(NH2):
                if attn_bf16:
                    p = pst([128, 128], BF16)
                    nc.tensor.transpose(p, kf[lt][:, t * 128:(t + 1) * 128],
                                        identt)
                else:
                    p = pst([128, 128], F32R)
                    nc.tensor.transpose(p, kf[lt][:, t * 128:(t + 1) * 128],
                                        identt)
                nc.vector.tensor_copy(
                    kfTz[2 * t][0:64, lt * 128:(lt + 1) * 128], p[0:64, :])
                nc.vector.tensor_copy(
                    kfTz[2 * t + 1][64:128, lt * 128:(lt + 1) * 128],
                    p[64:128, :])

        # ---------------- denominator via K1 = causal @ kf ----------------
        recip = [P.tile([128, 12], F32, tag=f"recip{lt}", name=f"recip{lt}")
                 for lt in range(LT)]
        for i in range(LT):
            ka = pst([128, 384])
            kb = pst([128, 384])
            for j in range(i + 1):
                m = ones128 if j < i else maskd
                nc.tensor.matmul(ka, m, kf[j][:, 0:384],
                                 start=(j == 0), stop=(j == i))
                nc.tensor.matmul(kb, m, kf[j][:, 384:768],
                                 start=(j == 0), stop=(j == i))
            dn = dn_p.tile([128, H * F], F32, tag="dn", name="dn")
            nc.vector.tensor_mul(dn[:, 0:384], qf[i][:, 0:384], ka)
            nc.vector.tensor_mul(dn[:, 384:768], qf[i][:, 384:768], kb)
            den = sm_p.tile([128, 12], F32, tag="den", name="den")
            nc.vector.reduce_sum(den, dn.rearrange("p (h f) -> p h f", f=64),
                                 axis=mybir.AxisListType.X)
            nc.vector.tensor_scalar_add(den, den, EPS)
            nc.vector.reciprocal(recip[i], den)
            for h in range(H):
                nc.gpsimd.tensor_scalar_mul(qf[i][:, h * 64:(h + 1) * 64],
                                            qf[i][:, h * 64:(h + 1) * 64],
                                            recip[i][:, h:h + 1])

        if phases < 5:
            raise _PhaseCut
        # ---------------- transpose qf -> [f, l] pairs ----------------
        # qfT[t] paired: rows 0:64 = head 2t, rows 64:128 = head 2t+1
        qfT = [P.tile([128, L], AD, tag=f"qfT{t}", name=f"qfT{t}")
               for t in range(NH2)]
        for lt in range(LT):
            for t in range(NH2):
                if attn_bf16:
                    p = pst([128, 128], BF16)
                    nc.tensor.transpose(p, qf[lt][:, t * 128:(t + 1) * 128],
                                        identt)
                else:
                    p = pst([128, 128], F32)
                    nc.tensor.transpose(p, qf[lt][:, t * 128:(t + 1) * 128],
                                        ident)
                nc.vector.tensor_copy(qfT[t][:, lt * 128:(lt + 1) * 128], p)

        if phases < 6:
            raise _PhaseCut
        # ---------------- scores ST[j,i] = kf @ qfT (causal) ----------------
        # ST_sb[h][j] covers i-columns [j*128, 512) ; diagonal block masked
        ST_sb = [[None] * LT for _ in range(H)]
        aT_all = [P.tile([128, L], AD, tag=f"aT{t}", name=f"aT{t}")
                  for t in range(NH2)]
        for t in range(NH2):
            pa = pst([128, L])
            for hh in range(2):
                h = 2 * t + hh
                for j in range(LT):
                    n = L - j * 128
                    pq = pst([128, n])
                    nc.tensor.matmul(
                        pq,
                        kfTz[h][:, j * 128:(j + 1) * 128],
                        qfT[t][:, j * 128:L],
                        start=True, stop=True)
                    st = st_p.tile([128, n], AD, tag="st", name="st")
                    nc.vector.tensor_mul(st[:, 0:128], pq[:, 0:128], maskd)
                    if n > 128:
                        nc.scalar.copy(st[:, 128:n], pq[:, 128:n])
                    ST_sb[h][j] = st
            for j in range(LT):
                for hh in range(2):
                    h = 2 * t + hh
                    nc.tensor.matmul(
                        pa[:, j * 128:L],
                        v_pboth[j][:, h * 128:(h + 1) * 128],
                        ST_sb[h][j],
                        start=(j == 0 and hh == 0),
                        stop=(j == LT - 1 and hh == 1))
            nc.vector.tensor_copy(aT_all[t], pa)

        if phases < 7:
            raise _PhaseCut
        # ---------------- output projection ----------------
        wo = wo_b if attn_bf16 else w_out_sb
        for lt in range(LT):
            po = [pst([128, 384]) for _ in range(2)]
            for et in range(ET):
                lhsT = aT_all[et][:, lt * 128:(lt + 1) * 128]
                for nh in range(2):
                    nc.tensor.matmul(po[nh], lhsT,
                                     wo[et][:, nh * 384:(nh + 1) * 384],
                                     start=(et == 0), stop=(et == ET - 1))
            osb = osb_p.tile([128, E], F32, tag="osb", name="osb")
            for nh in range(2):
                nc.vector.tensor_add(osb[:, nh * 384:(nh + 1) * 384], po[nh],
                                     b_out_sb[:, nh * 384:(nh + 1) * 384])
            nc.sync.dma_start(out=out_d[lt * 128:(lt + 1) * 128, :], in_=osb)
      except _PhaseCutE:
        pass

    if fix_waits:
        _fix_waits(nc)
    return nc


_CACHE = {}


def _get_nc():
    if "nc" not in _CACHE:
        _CACHE["nc"] = build_nc()
    return _CACHE["nc"]


def _host_consts(attn_bf16=ATTN_BF16):
    import ml_dtypes
    ad = ml_dtypes.bfloat16 if attn_bf16 else np.float32
    ident = np.eye(128, dtype=np.float32)
    return {
        "ident": ident,
        "ident_r": ident,
        "ident_a": ident.astype(ml_dtypes.bfloat16),
        "mask_diag": np.triu(np.ones((128, 128), dtype=np.float32)).astype(ad),
        "ones128": np.ones((128, 128), dtype=ad),
    }


def _in_maps(x, w_inp, b_inp, w_out, b_out, omega):
    f = lambda a: np.ascontiguousarray(np.asarray(a), dtype=np.float32)
    x, w_inp, b_inp = f(x), f(w_inp), f(b_inp)
    w_out, b_out, omega = f(w_out), f(b_out), f(omega)
    consts = _host_consts()
    maps = []
    for c in range(B):
        m = {"x": x[c], "w_inp": w_inp[0], "b_inp": b_inp,
             "w_out": w_out[0], "b_out": b_out, "omega": omega}
        m.update(consts)
        maps.append(m)
    return maps


def kernel(x, w_inp, b_inp, w_out, b_out, omega):
    nc = _get_nc()
    maps = _in_maps(x, w_inp, b_inp, w_out, b_out, omega)
    res = bass_utils.run_bass_kernel_spmd(nc, maps, core_ids=list(range(B)))
    return np.stack([res.results[c]["out"] for c in range(B)])


def run_traced(x, w_inp, b_inp, w_out, b_out, omega):
    """kernel() + HW time estimate. NTFF tracing is unavailable under this
    axon deployment, so time by wall-clock deltas on repeated dispatches."""
    import time
    from concourse import bass2jax
    nc = _get_nc()
    maps = _in_maps(x, w_inp, b_inp, w_out, b_out, omega)
    res = bass_utils.run_bass_kernel_spmd(nc, maps, core_ids=list(range(B)))
    out = np.stack([res.results[c]["out"] for c in range(B)])
    times = []
    for _ in range(6):
        t0 = time.perf_counter()
        bass2jax.run_bass_via_pjrt(nc, maps, n_cores=B)
        times.append(time.perf_counter() - t0)
    exec_ns = int(min(times) * 1e9)
    return out, exec_ns
